# revision 14
# baseline (speedup 1.0000x reference)
"""LIF spiking-neuron recurrence on Trainium2 (8 NeuronCores).

Reference semantics (TAU=1, THRESH=1, f32):
    mem = 0
    for t in range(T):
        mem = mem + x[t]
        spike[t] = (mem >= 1.0) ? 1.0 : 0.0
        mem = mem * (1 - spike[t])        # hard reset

Sharding: data-parallel over the batch axis (B=128 -> 16 rows/core).
Per-core layout: the [T, 16, 16384] shard is viewed as [T, 128, 2048]
(partition-major within a timestep slab) and pre-transposed on the host
to [128, T, 2048] so each partition's DMA runs are contiguous.

Engine mapping per timestep (tile [128, 2048] f32):
    DVE : tmp = mem + x_t            (tensor_tensor add, 1x, ~2.29us)
    ACT : d = Sqrt(tmp + (-1))       (NaN iff tmp < 1; affine is exact)
    ACT : spike = Is_finite(d)       (exact 1.0/0.0, written as bf16)
    DVE : mem = (tmp < 1) * tmp      (scalar_tensor_tensor, 1x, ~2.29us)
The ACT spike route was probed exact on HW for all threshold edge
cases (ties, +-1ulp); GpSimd is kept idle (f32 elementwise there runs
~15-30x below DVE and its shared-port lock stalls DVE). Spikes are
stored as uint8 (0/1 exact, probed) cutting store traffic 4x; the
host upcasts. DMAs are HWDGE (loads on SP ring, stores on ACT ring),
CHUNK timesteps per transfer; the first group loads per-step (1MB) so
compute starts early, and the last group stores per-step to shorten
the tail. Step 0 uses x_0 directly (mem starts at 0); the final
step's reset is dead code and skipped.
"""

import numpy as np

try:
    import concourse  # noqa: F401
except ImportError:  # pragma: no cover
    import sys

    for _p in ("/opt/trn_rl_repo", "/root/.axon_site/_ro/trn_rl_repo"):
        if _p not in sys.path:
            sys.path.insert(0, _p)

from concourse import bacc, mybir
from concourse.bass_utils import run_bass_kernel_spmd
from concourse.mybir import ActivationFunctionType as AF
from concourse.mybir import AluOpType
from concourse.tile import TileContext

T, B, D = 64, 128, 16384
NCORES = 8
BL = B // NCORES  # 16 batch rows per core
P = 128  # SBUF partitions
F = (BL * D) // P  # 2048 free elements per timestep slab
CHUNK = 4  # timesteps per DMA transfer


def build_nc(
    t_steps=T, f_free=F, chunk=CHUNK, x_bufs=4, s_bufs=3, t_bufs=4, d_bufs=1
):
    """Build + compile the per-core Bass program (identical on all cores)."""
    assert t_steps % chunk == 0
    f32 = mybir.dt.float32
    u8 = mybir.dt.uint8
    nc = bacc.Bacc(
        "TRN2", target_bir_lowering=False, debug=False, num_devices=NCORES
    )
    x_ext = nc.dram_tensor("x", [P, t_steps, f_free], f32, kind="ExternalInput")
    out_ext = nc.dram_tensor(
        "out", [P, t_steps, f_free], u8, kind="ExternalOutput"
    )
    n_groups = t_steps // chunk
    with TileContext(nc) as tc:
        with (
            tc.tile_pool(name="xp", bufs=x_bufs) as xp,
            tc.tile_pool(name="sp", bufs=s_bufs) as sp,
            tc.tile_pool(name="tp", bufs=t_bufs) as tp,
            tc.tile_pool(name="dp", bufs=d_bufs) as dp,
            tc.tile_pool(name="mp", bufs=1) as mp,
        ):
            mem = mp.tile([P, f_free], f32)
            bm1 = mp.tile([P, 1], f32, name="bm1")
            nc.vector.memset(bm1[:], -1.0)
            for g in range(n_groups):
                xt = xp.tile([P, chunk * f_free], f32, name="xt")
                xv = x_ext[:, g * chunk : (g + 1) * chunk, :]
                # per-step loads: slice-level deps let each TT start as
                # soon as its own 1MB lands instead of the whole 4MB.
                # The very first load is further quartered so step-0
                # compute starts after ~256KB.
                for j in range(chunk):
                    if g == 0 and j == 0:
                        q = f_free // 4
                        for k in range(4):
                            nc.sync.dma_start(
                                xt[:, k * q : (k + 1) * q], xv[:, 0, k * q : (k + 1) * q]
                            )
                    else:
                        nc.sync.dma_start(
                            xt[:, j * f_free : (j + 1) * f_free], xv[:, j, :]
                        )
                spk = sp.tile([P, chunk * f_free], u8, name="spk")
                for j in range(chunk):
                    t = g * chunk + j
                    xs = xt[:, j * f_free : (j + 1) * f_free]
                    ss = spk[:, j * f_free : (j + 1) * f_free]
                    if t == 0:
                        # mem==0: pre-reset membrane is just x_0; process
                        # in quarters so compute starts after 256KB lands
                        q = f_free // 4
                        for k in range(4):
                            xq = xt[:, k * q : (k + 1) * q]
                            d = dp.tile([P, f_free], f32, name="d")
                            nc.scalar.activation(
                                d[:, :q], xq, AF.Sqrt, bias=bm1[:], scale=1.0
                            )
                            nc.scalar.activation(
                                ss[:, k * q : (k + 1) * q], d[:, :q],
                                AF.Is_finite, bias=0.0, scale=1.0,
                            )
                            nc.vector.scalar_tensor_tensor(
                                mem[:, k * q : (k + 1) * q], xq, 1.0, xq,
                                AluOpType.is_lt, AluOpType.mult,
                            )
                        continue
                    tmp = tp.tile([P, f_free], f32, name="tmp")
                    nc.vector.tensor_tensor(tmp[:], mem[:], xs, AluOpType.add)
                    pre = tmp[:]
                    # spike = Is_finite(Sqrt(pre - 1)): NaN iff pre < 1
                    d = dp.tile([P, f_free], f32, name="d")
                    nc.scalar.activation(
                        d[:], pre, AF.Sqrt, bias=bm1[:], scale=1.0
                    )
                    nc.scalar.activation(
                        ss, d[:], AF.Is_finite, bias=0.0, scale=1.0
                    )
                    if t < t_steps - 1:  # last reset is dead code
                        nc.vector.scalar_tensor_tensor(
                            mem[:], pre, 1.0, pre, AluOpType.is_lt, AluOpType.mult
                        )
                    if g == n_groups - 1:
                        # per-step stores so the tail drains quickly
                        nc.scalar.dma_start(
                            out_ext[:, g * chunk + j, :], ss
                        )
                if g < n_groups - 1:
                    nc.scalar.dma_start(
                        out_ext[:, g * chunk : (g + 1) * chunk, :].rearrange(
                            "p t f -> p (t f)"
                        ),
                        spk[:],
                    )
    nc.compile()
    return nc


_cached_nc = None


def _get_nc():
    global _cached_nc
    if _cached_nc is None:
        _cached_nc = build_nc()
    return _cached_nc


def _shard(x):
    """Full [T, B, D] -> list of per-core [P, T, F] contiguous arrays."""
    in_maps = []
    for c in range(NCORES):
        xc = x[:, c * BL : (c + 1) * BL, :].reshape(T, P, F).transpose(1, 0, 2)
        in_maps.append({"x": np.ascontiguousarray(xc)})
    return in_maps


def _gather(results):
    """Per-core [P, T, F] uint8 outputs -> full [T, B, D] f32 (exact)."""
    outs = [
        np.asarray(results[c]["out"])
        .astype(np.float32)
        .transpose(1, 0, 2)
        .reshape(T, BL, D)
        for c in range(NCORES)
    ]
    return np.concatenate(outs, axis=1)


def run(x, trace=False, **kw):
    """Run on the 8 NeuronCores; returns (output, BassKernelResults)."""
    x = np.ascontiguousarray(np.asarray(x, dtype=np.float32))
    assert x.shape == (T, B, D), x.shape
    nc = _get_nc()
    res = run_bass_kernel_spmd(
        nc, _shard(x), core_ids=list(range(NCORES)), trace=trace, **kw
    )
    return _gather(res.results), res


def kernel(x: np.ndarray) -> np.ndarray:
    out, _ = run(x)
    return out


# revision 16
# speedup vs baseline: 1.0029x; 1.0029x over previous
"""LIF spiking-neuron recurrence on Trainium2 (8 NeuronCores).

Reference semantics (TAU=1, THRESH=1, f32):
    mem = 0
    for t in range(T):
        mem = mem + x[t]
        spike[t] = (mem >= 1.0) ? 1.0 : 0.0
        mem = mem * (1 - spike[t])        # hard reset

Sharding: data-parallel over the batch axis (B=128 -> 16 rows/core).
Per-core layout: the [T, 16, 16384] shard is viewed as [T, 128, 2048]
(partition-major within a timestep slab) and pre-transposed on the host
to [128, T, 2048] so each partition's DMA runs are contiguous.

Engine mapping per timestep (tile [128, 2048] f32):
    DVE : tmp = mem + x_t            (tensor_tensor add, 1x, ~2.29us)
    ACT : d = Sqrt(tmp + (-1))       (NaN iff tmp < 1; affine is exact)
    ACT : spike = Is_finite(d)       (exact 1.0/0.0, written as bf16)
    DVE : mem = (tmp < 1) * tmp      (scalar_tensor_tensor, 1x, ~2.29us)
The ACT spike route was probed exact on HW for all threshold edge
cases (ties, +-1ulp); GpSimd is kept idle (f32 elementwise there runs
~15-30x below DVE and its shared-port lock stalls DVE). Spikes are
stored as uint8 (0/1 exact, probed) cutting store traffic 4x; the
host upcasts. DMAs are HWDGE (loads on SP ring, stores on ACT ring),
CHUNK timesteps per transfer; the first group loads per-step (1MB) so
compute starts early, and the last group stores per-step to shorten
the tail. Step 0 uses x_0 directly (mem starts at 0); the final
step's reset is dead code and skipped.
"""

import numpy as np

try:
    import concourse  # noqa: F401
except ImportError:  # pragma: no cover
    import sys

    for _p in ("/opt/trn_rl_repo", "/root/.axon_site/_ro/trn_rl_repo"):
        if _p not in sys.path:
            sys.path.insert(0, _p)

from concourse import bacc, mybir
from concourse.bass_utils import run_bass_kernel_spmd
from concourse.mybir import ActivationFunctionType as AF
from concourse.mybir import AluOpType
from concourse.tile import TileContext

T, B, D = 64, 128, 16384
NCORES = 8
BL = B // NCORES  # 16 batch rows per core
P = 128  # SBUF partitions
F = (BL * D) // P  # 2048 free elements per timestep slab
CHUNK = 4  # timesteps per DMA transfer


def build_nc(
    t_steps=T, f_free=F, chunk=CHUNK, x_bufs=4, s_bufs=3, t_bufs=4, d_bufs=1
):
    """Build + compile the per-core Bass program (identical on all cores)."""
    assert t_steps % chunk == 0
    f32 = mybir.dt.float32
    u8 = mybir.dt.uint8
    nc = bacc.Bacc(
        "TRN2", target_bir_lowering=False, debug=False, num_devices=NCORES
    )
    x_ext = nc.dram_tensor("x", [P, t_steps, f_free], f32, kind="ExternalInput")
    out_ext = nc.dram_tensor(
        "out", [P, t_steps, f_free], u8, kind="ExternalOutput"
    )
    n_groups = t_steps // chunk
    with TileContext(nc) as tc:
        with (
            tc.tile_pool(name="xp", bufs=x_bufs) as xp,
            tc.tile_pool(name="sp", bufs=s_bufs) as sp,
            tc.tile_pool(name="tp", bufs=t_bufs) as tp,
            tc.tile_pool(name="dp", bufs=d_bufs) as dp,
            tc.tile_pool(name="mp", bufs=1) as mp,
        ):
            mem = mp.tile([P, f_free], f32)
            bm1 = mp.tile([P, 1], f32, name="bm1")
            nc.vector.memset(bm1[:], -1.0)
            for g in range(n_groups):
                xt = xp.tile([P, chunk * f_free], f32, name="xt")
                xv = x_ext[:, g * chunk : (g + 1) * chunk, :]
                # per-step loads: slice-level deps let each TT start as
                # soon as its own 1MB lands instead of the whole 4MB
                for j in range(chunk):
                    nc.sync.dma_start(
                        xt[:, j * f_free : (j + 1) * f_free], xv[:, j, :]
                    )
                spk = sp.tile([P, chunk * f_free], u8, name="spk")
                for j in range(chunk):
                    t = g * chunk + j
                    xs = xt[:, j * f_free : (j + 1) * f_free]
                    ss = spk[:, j * f_free : (j + 1) * f_free]
                    if t == 0:
                        pre = xs  # mem==0: pre-reset membrane is just x_0
                    else:
                        tmp = tp.tile([P, f_free], f32, name="tmp")
                        nc.vector.tensor_tensor(
                            tmp[:], mem[:], xs, AluOpType.add
                        )
                        pre = tmp[:]
                    # spike = Is_finite(Sqrt(pre - 1)): NaN iff pre < 1
                    d = dp.tile([P, f_free], f32, name="d")
                    nc.scalar.activation(
                        d[:], pre, AF.Sqrt, bias=bm1[:], scale=1.0
                    )
                    nc.scalar.activation(
                        ss, d[:], AF.Is_finite, bias=0.0, scale=1.0
                    )
                    if t < t_steps - 1:  # last reset is dead code
                        nc.vector.scalar_tensor_tensor(
                            mem[:], pre, 1.0, pre, AluOpType.is_lt, AluOpType.mult
                        )
                    if g == n_groups - 1:
                        # per-step stores so the tail drains quickly
                        nc.scalar.dma_start(
                            out_ext[:, g * chunk + j, :], ss
                        )
                if g < n_groups - 1:
                    nc.scalar.dma_start(
                        out_ext[:, g * chunk : (g + 1) * chunk, :].rearrange(
                            "p t f -> p (t f)"
                        ),
                        spk[:],
                    )
    nc.compile()
    return nc


_cached_nc = None


def _get_nc():
    global _cached_nc
    if _cached_nc is None:
        _cached_nc = build_nc()
    return _cached_nc


def _shard(x):
    """Full [T, B, D] -> list of per-core [P, T, F] contiguous arrays."""
    in_maps = []
    for c in range(NCORES):
        xc = x[:, c * BL : (c + 1) * BL, :].reshape(T, P, F).transpose(1, 0, 2)
        in_maps.append({"x": np.ascontiguousarray(xc)})
    return in_maps


def _gather(results):
    """Per-core [P, T, F] uint8 outputs -> full [T, B, D] f32 (exact)."""
    outs = [
        np.asarray(results[c]["out"])
        .astype(np.float32)
        .transpose(1, 0, 2)
        .reshape(T, BL, D)
        for c in range(NCORES)
    ]
    return np.concatenate(outs, axis=1)


def run(x, trace=False, **kw):
    """Run on the 8 NeuronCores; returns (output, BassKernelResults)."""
    x = np.ascontiguousarray(np.asarray(x, dtype=np.float32))
    assert x.shape == (T, B, D), x.shape
    nc = _get_nc()
    res = run_bass_kernel_spmd(
        nc, _shard(x), core_ids=list(range(NCORES)), trace=trace, **kw
    )
    return _gather(res.results), res


def kernel(x: np.ndarray) -> np.ndarray:
    out, _ = run(x)
    return out
